# revision 51
# baseline (speedup 1.0000x reference)
"""Trainium2 Bass kernel for nn_CrossAttention (sparse_attention).

Reference, per head h:
  a_1 = (q_1 @ k_2^T) * SCALE * mask_1     q_g = emb_g W_q + b_q
  a_2 = (k_1 @ q_2^T) * SCALE * mask_2     k_g = emb_g W_k + b_k
  mask_1[i,j] = nt1[i]==nt2[j], mask_2 = mask_1^T.

Algebra (host-side prep, exact): with M = S Wq Wk^T, g2 = S Wk bq,
g1 = S Wq bk, cc = S bq.bk:
  a_1      = (e1 M + 1 g2^T) e2^T + u1 1^T      u1 = e1 g1 + cc
  a_2^T    = (e2 M + 1 g2^T) e1^T + u2 1^T      u2 = e2 g1 + cc
Both matrices therefore have the SAME device shape: a host-projected
stationary operand times a raw embedding, plus a rank-1 row term that the
host adds during output assembly. Sorting rows by nt1-order (perm1) and
columns by nt2-order (perm2) makes both block-diagonal with identical
block geometry (c1[t] x c2[t]).

Sharding: one matrix x two heads per core (cores 0-3: a_1 for head pairs,
cores 4-7: a_2^T), a single SPMD program. Per core the device loads two
stationary arrays (s0/s1, head-specific) and ONE shared moving array, does
the block-diagonal score matmuls in bf16, and DMAs the bf16 blocks out.

Tail packing: a type's ragged tail rows (rows_t % 128) would cost a full
2*w-cycle matmul per head-unit. Both units share the type's column range,
so the host embeds the two tails side by side INSIDE the s0 layout (per
type: [s0 full tiles | s0 tail | s1 tail]; s1 ships full tiles only --
total input bytes unchanged) and ONE matmul computes both units' tail
rows whenever they fit a 128-row tile. ~10% less PE work, and the tail
compute/DMAs ride h0's input-arrival slack.

Off-block output stays zero via the runner's zero-initialized buffers; the
host converts, adds u, transposes a_2^T back, and scatters.
"""

import os
import numpy as np
import ml_dtypes

N = 2048
D = 256
H = 8
T = 5
SCALE = D ** (-0.5)
NCORES = 8
P = 128
C = D // P  # 2 contraction chunks

BF16 = ml_dtypes.bfloat16

# PE warm-up matmuls: burn the p-state ramp while input DMAs stream.
N_WARM = int(os.environ.get("K_WARM", "6"))

_PROG_CACHE: dict = {}


def _bounds(cnt):
    b = [0]
    for c in cnt:
        b.append(b[-1] + int(c))
    return b


def _geometry(c1):
    """Per-type full-tile/tail split and packed column offsets.

    s0 ships [fulls | s0-tail | s1-tail] per type (soff); s1 ships fulls
    only (foff). Total stationary bytes across s0+s1 are unchanged.
    """
    gf, rt, soff, foff = [], [], [], []
    so = fo = 0
    for t in range(T):
        rows = int(c1[t])
        g, r = rows // P, rows % P
        gf.append(g)
        rt.append(r)
        soff.append(so)
        foff.append(fo)
        so += g * P + 2 * r
        fo += g * P
    return gf, rt, soff, foff, so, fo


def _build_program(c1: tuple, c2: tuple):
    import concourse.bass as bass  # noqa: F401
    import concourse.mybir as mybir
    import concourse.tile as tile
    from concourse import bacc

    f32 = mybir.dt.float32
    b16 = mybir.dt.bfloat16

    nc = bacc.Bacc("TRN2", target_bir_lowering=False, debug=False,
                   num_devices=NCORES)

    b1 = _bounds(c1)
    b2 = _bounds(c2)
    gf, rt, soff, foff, SW, FW = _geometry(c1)

    widths = {"s0": SW, "s1": FW, "mov": N}
    e_dram = {
        v: nc.dram_tensor(v, [D, w], b16, kind="ExternalInput")
        for v, w in widths.items()
    }
    out_d = nc.dram_tensor("out", [2, N, N], b16, kind="ExternalOutput")

    with tile.TileContext(nc) as tc:
        with (
            tc.tile_pool(name="const", bufs=1) as constp,
            tc.tile_pool(name="emb", bufs=1) as embp,
            tc.tile_pool(name="stage", bufs=10) as stagep,
            tc.tile_pool(name="tstage", bufs=4) as tstagep,
            tc.tile_pool(name="pmm", bufs=4, space="PSUM") as psum_mm,
        ):
            # --- PE warm-up (p-state ramp) while input DMAs stream
            junk = constp.tile([P, 512], b16, tag="junk")
            nc.vector.memset(junk[:], 0.5)
            ps_w = psum_mm.tile([P, 2, 512], f32, tag="mm", name="mm")
            for _ in range(N_WARM):
                nc.tensor.matmul(ps_w[:, 0, :], junk[:, 0:P], junk[:],
                                 start=True, stop=True)

            e_sb = {
                v: embp.tile([P, C, w], b16, tag=v, name=v)
                for v, w in widths.items()
            }
            e_re = {
                v: e_dram[v].ap().rearrange("(c p) n -> p c n", p=P)
                for v in e_sb
            }
            # --- loads on the SP queue in consumption order: (s0, mov)
            # chunk pairs cut so h0's block k has its data at the earliest,
            # then s1 halves, then the small packed-tail array.
            scuts = [0] + [soff[t] + gf[t] * P + 2 * rt[t]
                           for t in range(T)]
            mcuts = [0] + [b2[t + 1] for t in range(T - 1)] + [N]
            for k in range(T):
                if scuts[k] < scuts[k + 1]:
                    # first chunk via SWDGE: Pool's issue path reaches the
                    # DMA engine ~180ns sooner than SP's SEQ+HWDGE chain
                    eng0 = nc.gpsimd if k == 0 else nc.sync
                    eng0.dma_start(
                        e_sb["s0"][:, :, scuts[k]:scuts[k + 1]],
                        e_re["s0"][:, :, scuts[k]:scuts[k + 1]])
                nc.sync.dma_start(e_sb["mov"][:, :, mcuts[k]:mcuts[k + 1]],
                                  e_re["mov"][:, :, mcuts[k]:mcuts[k + 1]])
            # first s1 half covers h1's first three processed blocks so
            # the type straddling a naive midpoint cut can't stall the PE
            szo = sorted(range(T), key=lambda t: -int(c1[t]) * int(c2[t]))
            half = max(foff[t] + gf[t] * P for t in szo[:3])
            for lo, hi in ((0, half), (half, FW)):
                if lo < hi:
                    nc.sync.dma_start(e_sb["s1"][:, :, lo:hi],
                                      e_re["s1"][:, :, lo:hi])

            ep = 0  # epilogue engine round-robin

            def epilogue(dst, src):
                # returns the DMA-capable engine whose queue the dependent
                # output DMA should ride (no cross-engine sem wait)
                nonlocal ep
                ep += 1
                if ep % 2 == 1:
                    nc.scalar.copy(dst, src)
                    return nc.scalar
                nc.vector.tensor_copy(dst, src)
                return nc.sync  # DVE can't DMA; SP is idle after inputs

            gt_max = max(gf) if max(gf) else 1

            def tail_block(t, c0, c1_, w):
                # both units' tail rows for type t in (at most) one matmul
                r = rt[t]
                tr0 = b1[t] + gf[t] * P
                stt = tstagep.tile([P, 512], b16, tag="tst", name="tst")
                tb = soff[t] + gf[t] * P  # tail columns inside s0
                units = ((0, 2 * r),) if 2 * r <= P else ((0, r), (1, r))
                for u0, span in units:
                    ps = psum_mm.tile([P, 2, 512], f32, tag="mm", name="mm")
                    for c in range(C):
                        nc.tensor.matmul(
                            ps[0:span, 0, 0:w],
                            e_sb["s0"][:, c,
                                       tb + u0 * r:tb + u0 * r + span],
                            e_sb["mov"][:, c, c0:c1_],
                            start=(c == 0),
                            stop=(c == C - 1),
                        )
                    epilogue(stt[u0 * r:u0 * r + span, 0:w],
                             ps[0:span, 0, 0:w])
                # tails ride the SWDGE queue: desc-gen on the otherwise-idle
                # Pool engine, not HWDGE
                for mat in range(2):
                    nc.gpsimd.dma_start(
                        out_d[mat, tr0:b1[t + 1], c0:c1_],
                        stt[mat * r:(mat + 1) * r, 0:w],
                    )

            def do_matrix(mat, vstat, order, split_last, with_tails):
                tt = e_sb[vstat]
                pieces = []
                for t in order:
                    for cc0 in range(b2[t], b2[t + 1], 512):
                        pieces.append((t, cc0, min(cc0 + 512, b2[t + 1])))
                for ti, (t, c0, c1_) in enumerate(pieces):
                    split = split_last and ti == len(pieces) - 1
                    w = c1_ - c0
                    g_t = gf[t]
                    st = stagep.tile([P, gt_max, 512], b16, tag="st",
                                     name="st")
                    pair_eng = []
                    for g0 in range(0, g_t, 2):
                        npair = min(2, g_t - g0)
                        ps = psum_mm.tile([P, 2, 512], f32, tag="mm",
                                          name="mm")
                        offs = soff if vstat == "s0" else foff
                        for g in range(g0, g0 + npair):
                            f0 = offs[t] + g * P
                            for c in range(C):
                                nc.tensor.matmul(
                                    ps[:, g - g0, 0:w],
                                    tt[:, c, f0:f0 + P],
                                    e_sb["mov"][:, c, c0:c1_],
                                    start=(c == 0),
                                    stop=(c == C - 1),
                                )
                        eng = epilogue(st[:, g0:g0 + npair, 0:w],
                                       ps[:, 0:npair, 0:w])
                        pair_eng.append(eng)
                        if split:
                            # last block: per-pair DMA right behind its
                            # epilogue; SWDGE skips the 565ns DMA_SEQ +
                            # HWDGE chain, reaching the engine sooner
                            r0 = b1[t] + g0 * P
                            dst = out_d[mat, r0:r0 + npair * P, c0:c1_]
                            nc.gpsimd.dma_start(
                                dst.rearrange("(g p) n -> p g n", p=P),
                                st[:, g0:g0 + npair, 0:w],
                            )
                    if not split and g_t:
                        eng = pair_eng[(g_t - 1) // 2]
                        dst = out_d[mat, b1[t]:b1[t] + g_t * P, c0:c1_]
                        eng.dma_start(
                            dst.rearrange("(g p) n -> p g n", p=P),
                            st[:, 0:g_t, 0:w],
                        )
                    # both units' packed tail rides with h0's pass, while
                    # this piece's mov columns are hot
                    if with_tails and rt[t]:
                        tail_block(t, c0, c1_, w)

            do_matrix(0, "s0", list(range(T)), False, True)
            # h1 (full tiles only now): big blocks first, smallest last so
            # the final transfers are small
            sz = sorted(range(T), key=lambda t: -int(c1[t]) * int(c2[t]))
            do_matrix(1, "s1", sz, True, False)

    nc.compile()
    return nc


def _get_program(c1, c2):
    key = (tuple(int(x) for x in c1), tuple(int(x) for x in c2))
    if key not in _PROG_CACHE:
        _PROG_CACHE[key] = _build_program(key[0], key[1])
    return _PROG_CACHE[key]


def kernel(emb_1, emb_2, node_type_1, node_type_2, W_q, b_q, W_k, b_k):
    from concourse.bass_utils import run_bass_kernel_spmd

    emb_1 = np.asarray(emb_1, dtype=np.float32)
    emb_2 = np.asarray(emb_2, dtype=np.float32)
    nt1 = np.asarray(node_type_1).astype(np.int64)
    nt2 = np.asarray(node_type_2).astype(np.int64)
    W_q = np.asarray(W_q, dtype=np.float32)
    W_k = np.asarray(W_k, dtype=np.float32)
    b_q = np.asarray(b_q, dtype=np.float32)
    b_k = np.asarray(b_k, dtype=np.float32)

    perm1 = np.argsort(nt1, kind="stable")
    perm2 = np.argsort(nt2, kind="stable")
    c1 = np.bincount(nt1, minlength=T)
    c2 = np.bincount(nt2, minlength=T)
    b1 = _bounds(c1)
    b2 = _bounds(c2)
    gf, rt, soff, foff, SW, FW = _geometry(c1)

    e1p1 = emb_1[perm1]          # a1 stationary source
    e2p1 = emb_2[perm1]          # a2^T stationary source
    mov1 = np.ascontiguousarray(emb_2[perm2].T.astype(BF16))  # a1 moving
    mov2 = np.ascontiguousarray(emb_1[perm2].T.astype(BF16))  # a2^T moving

    # per-head projection matrices / bias vectors
    Ms, g1s, g2s, ccs = [], [], [], []
    for h in range(H):
        sl = slice(h * D, (h + 1) * D)
        Wq, Wk = W_q[:, sl], W_k[:, sl]
        bq, bk = b_q[sl], b_k[sl]
        Ms.append(SCALE * (Wq @ Wk.T))
        g1s.append(SCALE * (Wq @ bk))
        g2s.append(SCALE * (Wk @ bq))
        ccs.append(float(SCALE * np.dot(bq, bk)))

    nc = _get_program(c1, c2)

    def compact(S):
        # full-tile columns only, per type
        return np.concatenate(
            [S[:, b1[t]:b1[t] + gf[t] * P] for t in range(T)], axis=1)

    def pack_s0(S0, S1):
        # per type: [S0 fulls | S0 tail | S1 tail]
        parts = []
        for t in range(T):
            fe = b1[t] + gf[t] * P
            parts.append(S0[:, b1[t]:fe])
            if rt[t]:
                parts.append(S0[:, fe:b1[t + 1]])
                parts.append(S1[:, fe:b1[t + 1]])
        return np.ascontiguousarray(np.concatenate(parts, axis=1))

    in_maps = []
    core_info = []  # (mat_kind, head0, head1, U0, U1)
    for mat, estat, eraw in ((0, e1p1, emb_1), (1, e2p1, emb_2)):
        for p in range(4):
            h0, h1 = 2 * p, 2 * p + 1
            S0 = np.ascontiguousarray(
                (estat @ Ms[h0] + g2s[h0]).T.astype(BF16))
            S1 = np.ascontiguousarray(
                (estat @ Ms[h1] + g2s[h1]).T.astype(BF16))
            U0 = (eraw @ g1s[h0] + ccs[h0])[perm1].astype(np.float32)
            U1 = (eraw @ g1s[h1] + ccs[h1])[perm1].astype(np.float32)
            im = {
                "s0": pack_s0(S0, S1), "s1": compact(S1),
                "mov": mov1 if mat == 0 else mov2,
            }
            in_maps.append(im)
            core_info.append((mat, h0, h1, U0, U1))

    res = run_bass_kernel_spmd(nc, in_maps, core_ids=list(range(NCORES)))

    out = np.empty((2 * H, N, N), dtype=np.float32)
    r1 = perm1[:, None]
    r2 = perm2[:, None]
    col1 = perm1[None, :]
    col2 = perm2[None, :]
    for core in range(NCORES):
        mat, h0, h1, U0, U1 = core_info[core]
        slabs = np.asarray(res.results[core]["out"]).astype(np.float32)
        for i, (h, U) in enumerate(((h0, U0), (h1, U1))):
            slab = slabs[i]
            for t in range(T):
                slab[b1[t]:b1[t + 1], b2[t]:b2[t + 1]] += \
                    U[b1[t]:b1[t + 1], None]
            if mat == 0:
                out[h][r1, col2] = slab
            else:
                out[H + h][r2, col1] = slab.T
    return out


# revision 52
# speedup vs baseline: 1.0371x; 1.0371x over previous
"""Trainium2 Bass kernel for nn_CrossAttention (sparse_attention).

Reference, per head h:
  a_1 = (q_1 @ k_2^T) * SCALE * mask_1     q_g = emb_g W_q + b_q
  a_2 = (k_1 @ q_2^T) * SCALE * mask_2     k_g = emb_g W_k + b_k
  mask_1[i,j] = nt1[i]==nt2[j], mask_2 = mask_1^T.

Algebra (host-side prep, exact): with M = S Wq Wk^T, g2 = S Wk bq,
g1 = S Wq bk, cc = S bq.bk:
  a_1      = (e1 M + 1 g2^T) e2^T + u1 1^T      u1 = e1 g1 + cc
  a_2^T    = (e2 M + 1 g2^T) e1^T + u2 1^T      u2 = e2 g1 + cc
Both matrices therefore have the SAME device shape: a host-projected
stationary operand times a raw embedding, plus a rank-1 row term that the
host adds during output assembly. Sorting rows by nt1-order (perm1) and
columns by nt2-order (perm2) makes both block-diagonal with identical
block geometry (c1[t] x c2[t]).

Sharding: one matrix x two heads per core (cores 0-3: a_1 for head pairs,
cores 4-7: a_2^T), a single SPMD program. Per core the device loads two
stationary arrays (s0/s1, head-specific) and ONE shared moving array, does
the block-diagonal score matmuls in bf16, and DMAs the bf16 blocks out.

Tail packing: a type's ragged tail rows (rows_t % 128) would cost a full
2*w-cycle matmul per head-unit. Both units share the type's column range,
so the host embeds the two tails side by side INSIDE the s0 layout (per
type: [s0 full tiles | s0 tail | s1 tail]; s1 ships full tiles only --
total input bytes unchanged) and ONE matmul computes both units' tail
rows whenever they fit a 128-row tile. ~10% less PE work, and the tail
compute/DMAs ride h0's input-arrival slack.

Off-block output stays zero via the runner's zero-initialized buffers; the
host converts, adds u, transposes a_2^T back, and scatters.
"""

import os
import numpy as np
import ml_dtypes

N = 2048
D = 256
H = 8
T = 5
SCALE = D ** (-0.5)
NCORES = 8
P = 128
C = D // P  # 2 contraction chunks

BF16 = ml_dtypes.bfloat16

# PE warm-up matmuls: burn the p-state ramp while input DMAs stream.
N_WARM = int(os.environ.get("K_WARM", "6"))

_PROG_CACHE: dict = {}


def _bounds(cnt):
    b = [0]
    for c in cnt:
        b.append(b[-1] + int(c))
    return b


def _geometry(c1):
    """Per-type full-tile/tail split and packed column offsets.

    s0 ships [fulls | s0-tail | s1-tail] per type (soff); s1 ships fulls
    only (foff). Total stationary bytes across s0+s1 are unchanged.
    """
    gf, rt, soff, foff = [], [], [], []
    so = fo = 0
    for t in range(T):
        rows = int(c1[t])
        g, r = rows // P, rows % P
        gf.append(g)
        rt.append(r)
        soff.append(so)
        foff.append(fo)
        so += g * P + 2 * r
        fo += g * P
    return gf, rt, soff, foff, so, fo


def _build_program(c1: tuple, c2: tuple):
    import concourse.bass as bass  # noqa: F401
    import concourse.mybir as mybir
    import concourse.tile as tile
    from concourse import bacc

    f32 = mybir.dt.float32
    b16 = mybir.dt.bfloat16

    nc = bacc.Bacc("TRN2", target_bir_lowering=False, debug=False,
                   num_devices=NCORES)

    b1 = _bounds(c1)
    b2 = _bounds(c2)
    gf, rt, soff, foff, SW, FW = _geometry(c1)

    widths = {"s0": SW, "s1": FW, "mov": N}
    e_dram = {
        v: nc.dram_tensor(v, [D, w], b16, kind="ExternalInput")
        for v, w in widths.items()
    }
    out_d = nc.dram_tensor("out", [2, N, N], b16, kind="ExternalOutput")

    with tile.TileContext(nc) as tc:
        with (
            tc.tile_pool(name="const", bufs=1) as constp,
            tc.tile_pool(name="emb", bufs=1) as embp,
            tc.tile_pool(name="stage", bufs=10) as stagep,
            tc.tile_pool(name="tstage", bufs=4) as tstagep,
            tc.tile_pool(name="pmm", bufs=4, space="PSUM") as psum_mm,
        ):
            # --- PE warm-up (p-state ramp) while input DMAs stream
            junk = constp.tile([P, 512], b16, tag="junk")
            nc.vector.memset(junk[:], 0.5)
            ps_w = psum_mm.tile([P, 2, 512], f32, tag="mm", name="mm")
            for _ in range(N_WARM):
                nc.tensor.matmul(ps_w[:, 0, :], junk[:, 0:P], junk[:],
                                 start=True, stop=True)

            e_sb = {
                v: embp.tile([P, C, w], b16, tag=v, name=v)
                for v, w in widths.items()
            }
            e_re = {
                v: e_dram[v].ap().rearrange("(c p) n -> p c n", p=P)
                for v in e_sb
            }
            # --- loads on the SP queue in consumption order: (s0, mov)
            # chunk pairs cut so h0's block k has its data at the earliest,
            # then s1 halves, then the small packed-tail array.
            scuts = [0] + [soff[t] + gf[t] * P + 2 * rt[t]
                           for t in range(T)]
            mcuts = [0] + [b2[t + 1] for t in range(T - 1)] + [N]
            for k in range(T):
                if scuts[k] < scuts[k + 1]:
                    # first chunk via SWDGE: Pool's issue path reaches the
                    # DMA engine ~180ns sooner than SP's SEQ+HWDGE chain
                    eng0 = nc.gpsimd if k == 0 else nc.sync
                    eng0.dma_start(
                        e_sb["s0"][:, :, scuts[k]:scuts[k + 1]],
                        e_re["s0"][:, :, scuts[k]:scuts[k + 1]])
                nc.sync.dma_start(e_sb["mov"][:, :, mcuts[k]:mcuts[k + 1]],
                                  e_re["mov"][:, :, mcuts[k]:mcuts[k + 1]])
            # first s1 half covers h1's first three processed blocks so
            # the type straddling a naive midpoint cut can't stall the PE
            szo = sorted(range(T), key=lambda t: -int(c1[t]) * int(c2[t]))
            half = max(foff[t] + gf[t] * P for t in szo[:3])
            for lo, hi in ((0, half), (half, FW)):
                if lo < hi:
                    nc.sync.dma_start(e_sb["s1"][:, :, lo:hi],
                                      e_re["s1"][:, :, lo:hi])

            ep = 0  # epilogue engine round-robin

            def epilogue(dst, src):
                # returns the DMA-capable engine whose queue the dependent
                # output DMA should ride (no cross-engine sem wait)
                nonlocal ep
                ep += 1
                if ep % 2 == 1:
                    nc.scalar.copy(dst, src)
                    return nc.scalar
                nc.vector.tensor_copy(dst, src)
                return nc.sync  # DVE can't DMA; SP is idle after inputs

            gt_max = max(gf) if max(gf) else 1

            def tail_block(t, c0, c1_, w):
                # both units' tail rows for type t in (at most) one matmul
                r = rt[t]
                tr0 = b1[t] + gf[t] * P
                stt = tstagep.tile([P, 512], b16, tag="tst", name="tst")
                tb = soff[t] + gf[t] * P  # tail columns inside s0
                units = ((0, 2 * r),) if 2 * r <= P else ((0, r), (1, r))
                for u0, span in units:
                    ps = psum_mm.tile([P, 2, 512], f32, tag="mm", name="mm")
                    for c in range(C):
                        nc.tensor.matmul(
                            ps[0:span, 0, 0:w],
                            e_sb["s0"][:, c,
                                       tb + u0 * r:tb + u0 * r + span],
                            e_sb["mov"][:, c, c0:c1_],
                            start=(c == 0),
                            stop=(c == C - 1),
                        )
                    epilogue(stt[u0 * r:u0 * r + span, 0:w],
                             ps[0:span, 0, 0:w])
                # tails ride the SWDGE queue: desc-gen on the otherwise-idle
                # Pool engine, not HWDGE
                for mat in range(2):
                    nc.gpsimd.dma_start(
                        out_d[mat, tr0:b1[t + 1], c0:c1_],
                        stt[mat * r:(mat + 1) * r, 0:w],
                    )

            def do_matrix(mat, vstat, order, split_last, with_tails):
                tt = e_sb[vstat]
                pieces = []
                for t in order:
                    for cc0 in range(b2[t], b2[t + 1], 512):
                        pieces.append((t, cc0, min(cc0 + 512, b2[t + 1])))
                for ti, (t, c0, c1_) in enumerate(pieces):
                    split = split_last and ti == len(pieces) - 1
                    w = c1_ - c0
                    g_t = gf[t]
                    st = stagep.tile([P, gt_max, 512], b16, tag="st",
                                     name="st")
                    pair_eng = []
                    for g0 in range(0, g_t, 2):
                        npair = min(2, g_t - g0)
                        ps = psum_mm.tile([P, 2, 512], f32, tag="mm",
                                          name="mm")
                        offs = soff if vstat == "s0" else foff
                        for g in range(g0, g0 + npair):
                            f0 = offs[t] + g * P
                            for c in range(C):
                                nc.tensor.matmul(
                                    ps[:, g - g0, 0:w],
                                    tt[:, c, f0:f0 + P],
                                    e_sb["mov"][:, c, c0:c1_],
                                    start=(c == 0),
                                    stop=(c == C - 1),
                                )
                        eng = epilogue(st[:, g0:g0 + npair, 0:w],
                                       ps[:, 0:npair, 0:w])
                        pair_eng.append(eng)
                        if split:
                            # last block: per-pair DMA right behind its
                            # epilogue, so the final transfer is tiny
                            r0 = b1[t] + g0 * P
                            dst = out_d[mat, r0:r0 + npair * P, c0:c1_]
                            eng.dma_start(
                                dst.rearrange("(g p) n -> p g n", p=P),
                                st[:, g0:g0 + npair, 0:w],
                            )
                    if not split and g_t:
                        eng = pair_eng[(g_t - 1) // 2]
                        dst = out_d[mat, b1[t]:b1[t] + g_t * P, c0:c1_]
                        eng.dma_start(
                            dst.rearrange("(g p) n -> p g n", p=P),
                            st[:, 0:g_t, 0:w],
                        )
                    # both units' packed tail rides with h0's pass, while
                    # this piece's mov columns are hot
                    if with_tails and rt[t]:
                        tail_block(t, c0, c1_, w)

            do_matrix(0, "s0", list(range(T)), False, True)
            # h1 (full tiles only now): big blocks first, smallest last so
            # the final transfers are small
            sz = sorted(range(T), key=lambda t: -int(c1[t]) * int(c2[t]))
            do_matrix(1, "s1", sz, True, False)

    nc.compile()
    return nc


def _get_program(c1, c2):
    key = (tuple(int(x) for x in c1), tuple(int(x) for x in c2))
    if key not in _PROG_CACHE:
        _PROG_CACHE[key] = _build_program(key[0], key[1])
    return _PROG_CACHE[key]


def kernel(emb_1, emb_2, node_type_1, node_type_2, W_q, b_q, W_k, b_k):
    from concourse.bass_utils import run_bass_kernel_spmd

    emb_1 = np.asarray(emb_1, dtype=np.float32)
    emb_2 = np.asarray(emb_2, dtype=np.float32)
    nt1 = np.asarray(node_type_1).astype(np.int64)
    nt2 = np.asarray(node_type_2).astype(np.int64)
    W_q = np.asarray(W_q, dtype=np.float32)
    W_k = np.asarray(W_k, dtype=np.float32)
    b_q = np.asarray(b_q, dtype=np.float32)
    b_k = np.asarray(b_k, dtype=np.float32)

    perm1 = np.argsort(nt1, kind="stable")
    perm2 = np.argsort(nt2, kind="stable")
    c1 = np.bincount(nt1, minlength=T)
    c2 = np.bincount(nt2, minlength=T)
    b1 = _bounds(c1)
    b2 = _bounds(c2)
    gf, rt, soff, foff, SW, FW = _geometry(c1)

    e1p1 = emb_1[perm1]          # a1 stationary source
    e2p1 = emb_2[perm1]          # a2^T stationary source
    mov1 = np.ascontiguousarray(emb_2[perm2].T.astype(BF16))  # a1 moving
    mov2 = np.ascontiguousarray(emb_1[perm2].T.astype(BF16))  # a2^T moving

    # per-head projection matrices / bias vectors
    Ms, g1s, g2s, ccs = [], [], [], []
    for h in range(H):
        sl = slice(h * D, (h + 1) * D)
        Wq, Wk = W_q[:, sl], W_k[:, sl]
        bq, bk = b_q[sl], b_k[sl]
        Ms.append(SCALE * (Wq @ Wk.T))
        g1s.append(SCALE * (Wq @ bk))
        g2s.append(SCALE * (Wk @ bq))
        ccs.append(float(SCALE * np.dot(bq, bk)))

    nc = _get_program(c1, c2)

    def compact(S):
        # full-tile columns only, per type
        return np.concatenate(
            [S[:, b1[t]:b1[t] + gf[t] * P] for t in range(T)], axis=1)

    def pack_s0(S0, S1):
        # per type: [S0 fulls | S0 tail | S1 tail]
        parts = []
        for t in range(T):
            fe = b1[t] + gf[t] * P
            parts.append(S0[:, b1[t]:fe])
            if rt[t]:
                parts.append(S0[:, fe:b1[t + 1]])
                parts.append(S1[:, fe:b1[t + 1]])
        return np.ascontiguousarray(np.concatenate(parts, axis=1))

    in_maps = []
    core_info = []  # (mat_kind, head0, head1, U0, U1)
    for mat, estat, eraw in ((0, e1p1, emb_1), (1, e2p1, emb_2)):
        for p in range(4):
            h0, h1 = 2 * p, 2 * p + 1
            S0 = np.ascontiguousarray(
                (estat @ Ms[h0] + g2s[h0]).T.astype(BF16))
            S1 = np.ascontiguousarray(
                (estat @ Ms[h1] + g2s[h1]).T.astype(BF16))
            U0 = (eraw @ g1s[h0] + ccs[h0])[perm1].astype(np.float32)
            U1 = (eraw @ g1s[h1] + ccs[h1])[perm1].astype(np.float32)
            im = {
                "s0": pack_s0(S0, S1), "s1": compact(S1),
                "mov": mov1 if mat == 0 else mov2,
            }
            in_maps.append(im)
            core_info.append((mat, h0, h1, U0, U1))

    res = run_bass_kernel_spmd(nc, in_maps, core_ids=list(range(NCORES)))

    out = np.empty((2 * H, N, N), dtype=np.float32)
    r1 = perm1[:, None]
    r2 = perm2[:, None]
    col1 = perm1[None, :]
    col2 = perm2[None, :]
    for core in range(NCORES):
        mat, h0, h1, U0, U1 = core_info[core]
        slabs = np.asarray(res.results[core]["out"]).astype(np.float32)
        for i, (h, U) in enumerate(((h0, U0), (h1, U1))):
            slab = slabs[i]
            for t in range(T):
                slab[b1[t]:b1[t + 1], b2[t]:b2[t + 1]] += \
                    U[b1[t]:b1[t + 1], None]
            if mat == 0:
                out[h][r1, col2] = slab
            else:
                out[H + h][r2, col1] = slab.T
    return out


# revision 53
# speedup vs baseline: 1.0457x; 1.0083x over previous
"""Trainium2 Bass kernel for nn_CrossAttention (sparse_attention).

Reference, per head h:
  a_1 = (q_1 @ k_2^T) * SCALE * mask_1     q_g = emb_g W_q + b_q
  a_2 = (k_1 @ q_2^T) * SCALE * mask_2     k_g = emb_g W_k + b_k
  mask_1[i,j] = nt1[i]==nt2[j], mask_2 = mask_1^T.

Algebra (host-side prep, exact): with M = S Wq Wk^T, g2 = S Wk bq,
g1 = S Wq bk, cc = S bq.bk:
  a_1      = (e1 M + 1 g2^T) e2^T + u1 1^T      u1 = e1 g1 + cc
  a_2^T    = (e2 M + 1 g2^T) e1^T + u2 1^T      u2 = e2 g1 + cc
Both matrices therefore have the SAME device shape: a host-projected
stationary operand times a raw embedding, plus a rank-1 row term that the
host adds during output assembly. Sorting rows by nt1-order (perm1) and
columns by nt2-order (perm2) makes both block-diagonal with identical
block geometry (c1[t] x c2[t]).

Sharding: one matrix x two heads per core (cores 0-3: a_1 for head pairs,
cores 4-7: a_2^T), a single SPMD program. Per core the device loads two
stationary arrays (s0/s1, head-specific) and ONE shared moving array, does
the block-diagonal score matmuls in bf16, and DMAs the bf16 blocks out.

Tail packing: a type's ragged tail rows (rows_t % 128) would cost a full
2*w-cycle matmul per head-unit. Both units share the type's column range,
so the host embeds the two tails side by side INSIDE the s0 layout (per
type: [s0 full tiles | s0 tail | s1 tail]; s1 ships full tiles only --
total input bytes unchanged) and ONE matmul computes both units' tail
rows whenever they fit a 128-row tile. ~10% less PE work, and the tail
compute/DMAs ride h0's input-arrival slack.

Off-block output stays zero via the runner's zero-initialized buffers; the
host converts, adds u, transposes a_2^T back, and scatters.
"""

import os
import numpy as np
import ml_dtypes

N = 2048
D = 256
H = 8
T = 5
SCALE = D ** (-0.5)
NCORES = 8
P = 128
C = D // P  # 2 contraction chunks

BF16 = ml_dtypes.bfloat16

# PE warm-up matmuls: burn the p-state ramp while input DMAs stream.
N_WARM = int(os.environ.get("K_WARM", "6"))

_PROG_CACHE: dict = {}


def _bounds(cnt):
    b = [0]
    for c in cnt:
        b.append(b[-1] + int(c))
    return b


def _geometry(c1):
    """Per-type full-tile/tail split and packed column offsets.

    s0 ships [fulls | s0-tail | s1-tail] per type (soff); s1 ships fulls
    only (foff). Total stationary bytes across s0+s1 are unchanged.
    """
    gf, rt, soff, foff = [], [], [], []
    so = fo = 0
    for t in range(T):
        rows = int(c1[t])
        g, r = rows // P, rows % P
        gf.append(g)
        rt.append(r)
        soff.append(so)
        foff.append(fo)
        so += g * P + 2 * r
        fo += g * P
    return gf, rt, soff, foff, so, fo


def _build_program(c1: tuple, c2: tuple):
    import concourse.bass as bass  # noqa: F401
    import concourse.mybir as mybir
    import concourse.tile as tile
    from concourse import bacc

    f32 = mybir.dt.float32
    b16 = mybir.dt.bfloat16

    nc = bacc.Bacc("TRN2", target_bir_lowering=False, debug=False,
                   num_devices=NCORES)

    b1 = _bounds(c1)
    b2 = _bounds(c2)
    gf, rt, soff, foff, SW, FW = _geometry(c1)

    widths = {"s0": SW, "s1": FW, "mov": N}
    e_dram = {
        v: nc.dram_tensor(v, [D, w], b16, kind="ExternalInput")
        for v, w in widths.items()
    }
    out_d = nc.dram_tensor("out", [2, N, N], b16, kind="ExternalOutput")

    with tile.TileContext(nc) as tc:
        with (
            tc.tile_pool(name="const", bufs=1) as constp,
            tc.tile_pool(name="emb", bufs=1) as embp,
            tc.tile_pool(name="stage", bufs=10) as stagep,
            tc.tile_pool(name="tstage", bufs=4) as tstagep,
            tc.tile_pool(name="pmm", bufs=4, space="PSUM") as psum_mm,
        ):
            # --- PE warm-up (p-state ramp) while input DMAs stream
            junk = constp.tile([P, 512], b16, tag="junk")
            nc.vector.memset(junk[:], 0.5)
            ps_w = psum_mm.tile([P, 2, 512], f32, tag="mm", name="mm")
            for _ in range(N_WARM):
                nc.tensor.matmul(ps_w[:, 0, :], junk[:, 0:P], junk[:],
                                 start=True, stop=True)

            e_sb = {
                v: embp.tile([P, C, w], b16, tag=v, name=v)
                for v, w in widths.items()
            }
            e_re = {
                v: e_dram[v].ap().rearrange("(c p) n -> p c n", p=P)
                for v in e_sb
            }
            # --- loads on the SP queue in consumption order: (s0, mov)
            # chunk pairs cut so h0's block k has its data at the earliest,
            # then s1 halves, then the small packed-tail array.
            scuts = [0] + [soff[t] + gf[t] * P + 2 * rt[t]
                           for t in range(T)]
            mcuts = [0] + [b2[t + 1] for t in range(T - 1)] + [N]
            for k in range(T):
                if scuts[k] < scuts[k + 1]:
                    # first chunk via SWDGE: Pool's issue path reaches the
                    # DMA engine ~180ns sooner than SP's SEQ+HWDGE chain
                    eng0 = nc.gpsimd if k == 0 else nc.sync
                    eng0.dma_start(
                        e_sb["s0"][:, :, scuts[k]:scuts[k + 1]],
                        e_re["s0"][:, :, scuts[k]:scuts[k + 1]])
                nc.sync.dma_start(e_sb["mov"][:, :, mcuts[k]:mcuts[k + 1]],
                                  e_re["mov"][:, :, mcuts[k]:mcuts[k + 1]])
            # first s1 half covers h1's first three processed blocks so
            # the type straddling a naive midpoint cut can't stall the PE
            szo = sorted(range(T), key=lambda t: -int(c1[t]) * int(c2[t]))
            half = max(foff[t] + gf[t] * P for t in szo[:3])
            for lo, hi in ((0, half), (half, FW)):
                if lo < hi:
                    nc.sync.dma_start(e_sb["s1"][:, :, lo:hi],
                                      e_re["s1"][:, :, lo:hi])

            ep = 0  # epilogue engine round-robin

            def epilogue(dst, src):
                # returns the DMA-capable engine whose queue the dependent
                # output DMA should ride (no cross-engine sem wait)
                nonlocal ep
                ep += 1
                if ep % 2 == 1:
                    nc.scalar.copy(dst, src)
                    return nc.scalar
                nc.vector.tensor_copy(dst, src)
                return nc.sync  # DVE can't DMA; SP is idle after inputs

            gt_max = max(gf) if max(gf) else 1

            def tail_block(t, c0, c1_, w):
                # both units' tail rows for type t in (at most) one matmul
                r = rt[t]
                tr0 = b1[t] + gf[t] * P
                stt = tstagep.tile([P, 512], b16, tag="tst", name="tst")
                tb = soff[t] + gf[t] * P  # tail columns inside s0
                units = ((0, 2 * r),) if 2 * r <= P else ((0, r), (1, r))
                for u0, span in units:
                    ps = psum_mm.tile([P, 2, 512], f32, tag="mm", name="mm")
                    for c in range(C):
                        nc.tensor.matmul(
                            ps[0:span, 0, 0:w],
                            e_sb["s0"][:, c,
                                       tb + u0 * r:tb + u0 * r + span],
                            e_sb["mov"][:, c, c0:c1_],
                            start=(c == 0),
                            stop=(c == C - 1),
                        )
                    epilogue(stt[u0 * r:u0 * r + span, 0:w],
                             ps[0:span, 0, 0:w])
                # tails ride the SWDGE queue: desc-gen on the otherwise-idle
                # Pool engine, not HWDGE
                for mat in range(2):
                    nc.gpsimd.dma_start(
                        out_d[mat, tr0:b1[t + 1], c0:c1_],
                        stt[mat * r:(mat + 1) * r, 0:w],
                    )

            def do_matrix(mat, vstat, order, split_last, with_tails):
                tt = e_sb[vstat]
                pieces = []
                for t in order:
                    for cc0 in range(b2[t], b2[t + 1], 512):
                        pieces.append((t, cc0, min(cc0 + 512, b2[t + 1])))
                for ti, (t, c0, c1_) in enumerate(pieces):
                    split = split_last and ti == len(pieces) - 1
                    w = c1_ - c0
                    g_t = gf[t]
                    st = stagep.tile([P, gt_max, 512], b16, tag="st",
                                     name="st")
                    pair_eng = []
                    for g0 in range(0, g_t, 2):
                        npair = min(2, g_t - g0)
                        ps = psum_mm.tile([P, 2, 512], f32, tag="mm",
                                          name="mm")
                        offs = soff if vstat == "s0" else foff
                        for g in range(g0, g0 + npair):
                            f0 = offs[t] + g * P
                            for c in range(C):
                                nc.tensor.matmul(
                                    ps[:, g - g0, 0:w],
                                    tt[:, c, f0:f0 + P],
                                    e_sb["mov"][:, c, c0:c1_],
                                    start=(c == 0),
                                    stop=(c == C - 1),
                                )
                        eng = epilogue(st[:, g0:g0 + npair, 0:w],
                                       ps[:, 0:npair, 0:w])
                        pair_eng.append(eng)
                        if split:
                            # last block: per-pair DMA; invert the queue
                            # mapping -- SP's SEQ is idle here and its DGE
                            # delay is 134ns shorter than ACT's
                            eng = nc.sync if eng is nc.scalar else nc.scalar
                            r0 = b1[t] + g0 * P
                            dst = out_d[mat, r0:r0 + npair * P, c0:c1_]
                            eng.dma_start(
                                dst.rearrange("(g p) n -> p g n", p=P),
                                st[:, g0:g0 + npair, 0:w],
                            )
                    if not split and g_t:
                        eng = pair_eng[(g_t - 1) // 2]
                        dst = out_d[mat, b1[t]:b1[t] + g_t * P, c0:c1_]
                        eng.dma_start(
                            dst.rearrange("(g p) n -> p g n", p=P),
                            st[:, 0:g_t, 0:w],
                        )
                    # both units' packed tail rides with h0's pass, while
                    # this piece's mov columns are hot
                    if with_tails and rt[t]:
                        tail_block(t, c0, c1_, w)

            do_matrix(0, "s0", list(range(T)), False, True)
            # h1 (full tiles only now): big blocks first, smallest last so
            # the final transfers are small
            sz = sorted(range(T), key=lambda t: -int(c1[t]) * int(c2[t]))
            do_matrix(1, "s1", sz, True, False)

    nc.compile()
    return nc


def _get_program(c1, c2):
    key = (tuple(int(x) for x in c1), tuple(int(x) for x in c2))
    if key not in _PROG_CACHE:
        _PROG_CACHE[key] = _build_program(key[0], key[1])
    return _PROG_CACHE[key]


def kernel(emb_1, emb_2, node_type_1, node_type_2, W_q, b_q, W_k, b_k):
    from concourse.bass_utils import run_bass_kernel_spmd

    emb_1 = np.asarray(emb_1, dtype=np.float32)
    emb_2 = np.asarray(emb_2, dtype=np.float32)
    nt1 = np.asarray(node_type_1).astype(np.int64)
    nt2 = np.asarray(node_type_2).astype(np.int64)
    W_q = np.asarray(W_q, dtype=np.float32)
    W_k = np.asarray(W_k, dtype=np.float32)
    b_q = np.asarray(b_q, dtype=np.float32)
    b_k = np.asarray(b_k, dtype=np.float32)

    perm1 = np.argsort(nt1, kind="stable")
    perm2 = np.argsort(nt2, kind="stable")
    c1 = np.bincount(nt1, minlength=T)
    c2 = np.bincount(nt2, minlength=T)
    b1 = _bounds(c1)
    b2 = _bounds(c2)
    gf, rt, soff, foff, SW, FW = _geometry(c1)

    e1p1 = emb_1[perm1]          # a1 stationary source
    e2p1 = emb_2[perm1]          # a2^T stationary source
    mov1 = np.ascontiguousarray(emb_2[perm2].T.astype(BF16))  # a1 moving
    mov2 = np.ascontiguousarray(emb_1[perm2].T.astype(BF16))  # a2^T moving

    # per-head projection matrices / bias vectors
    Ms, g1s, g2s, ccs = [], [], [], []
    for h in range(H):
        sl = slice(h * D, (h + 1) * D)
        Wq, Wk = W_q[:, sl], W_k[:, sl]
        bq, bk = b_q[sl], b_k[sl]
        Ms.append(SCALE * (Wq @ Wk.T))
        g1s.append(SCALE * (Wq @ bk))
        g2s.append(SCALE * (Wk @ bq))
        ccs.append(float(SCALE * np.dot(bq, bk)))

    nc = _get_program(c1, c2)

    def compact(S):
        # full-tile columns only, per type
        return np.concatenate(
            [S[:, b1[t]:b1[t] + gf[t] * P] for t in range(T)], axis=1)

    def pack_s0(S0, S1):
        # per type: [S0 fulls | S0 tail | S1 tail]
        parts = []
        for t in range(T):
            fe = b1[t] + gf[t] * P
            parts.append(S0[:, b1[t]:fe])
            if rt[t]:
                parts.append(S0[:, fe:b1[t + 1]])
                parts.append(S1[:, fe:b1[t + 1]])
        return np.ascontiguousarray(np.concatenate(parts, axis=1))

    in_maps = []
    core_info = []  # (mat_kind, head0, head1, U0, U1)
    for mat, estat, eraw in ((0, e1p1, emb_1), (1, e2p1, emb_2)):
        for p in range(4):
            h0, h1 = 2 * p, 2 * p + 1
            S0 = np.ascontiguousarray(
                (estat @ Ms[h0] + g2s[h0]).T.astype(BF16))
            S1 = np.ascontiguousarray(
                (estat @ Ms[h1] + g2s[h1]).T.astype(BF16))
            U0 = (eraw @ g1s[h0] + ccs[h0])[perm1].astype(np.float32)
            U1 = (eraw @ g1s[h1] + ccs[h1])[perm1].astype(np.float32)
            im = {
                "s0": pack_s0(S0, S1), "s1": compact(S1),
                "mov": mov1 if mat == 0 else mov2,
            }
            in_maps.append(im)
            core_info.append((mat, h0, h1, U0, U1))

    res = run_bass_kernel_spmd(nc, in_maps, core_ids=list(range(NCORES)))

    out = np.empty((2 * H, N, N), dtype=np.float32)
    r1 = perm1[:, None]
    r2 = perm2[:, None]
    col1 = perm1[None, :]
    col2 = perm2[None, :]
    for core in range(NCORES):
        mat, h0, h1, U0, U1 = core_info[core]
        slabs = np.asarray(res.results[core]["out"]).astype(np.float32)
        for i, (h, U) in enumerate(((h0, U0), (h1, U1))):
            slab = slabs[i]
            for t in range(T):
                slab[b1[t]:b1[t + 1], b2[t]:b2[t + 1]] += \
                    U[b1[t]:b1[t + 1], None]
            if mat == 0:
                out[h][r1, col2] = slab
            else:
                out[H + h][r2, col1] = slab.T
    return out
